# revision 31
# baseline (speedup 1.0000x reference)
"""DispersionLoss kernel for Trainium2 (8 NeuronCores, Bass/Tile).

Reference computation (N=16384, F=64, K=32, C=128):
    bin_mass[f,k]  = sum_n m[n,f,k] + EPS
    SWY[f,k,c]     = sum_n m[n,f,k] * y[n,c]
    cent[f,k,c]    = SWY / bin_mass
    loss_dispersion= sum_fk ( A/bin_mass - c_sq - EPS*c_sq/bin_mass )
        where A[f,k] = sum_n m[n,f,k]*|y_n|^2   (algebraic expansion: the
        cross term sum_n m*cross equals bin_mass*c_sq exactly)
    loss_entropy   = sum_fk p*log(p+EPS), p = bin_mass/N
    loss_repulsion = sum_f sum_k exp(-|cent[f,k]-cent[f,k+1]|^2)
    loss_inter     = sum_f sum_{k<j} exp(-|cent[f,k]-cent[f,j]|^2) / F

Sharding: over F (8 features per core); every loss term decomposes per-f.

Device phase (the N-reduction, 99.9% of FLOPs): per 128-row subtile s, the
G half-tiles (128n x 128fk) are the STATIONARY operands and the moving
operand is yext_s = [Y | 1 | ysq_hi | ysq_lo] (128n x 132).  Two matmuls
per subtile accumulate ps_h[fk, 132] = [SWY | mass | A_hi | A_lo] directly
in bin-major layout.  y_sq is computed exactly on the host from f32 y and
shipped split into fp8 hi+lo parts so its quantization error is ~1e-4.

G and Y are host-interleaved into ONE dram tensor in exact consumption
order, so a single DMA ring delivers them with no cross-stream contention:
block b = [g(s0) g(s1) y(s0) y(s1)] x 4 subtile-pairs (all slots 16B-
aligned for fast weight load), 410 KB, whose transfer time (~1.15 us)
matches the matmul consumption per block.

The per-core (256 x 132) f32 result is DMA'd out; the host finishes the
tiny (F,K,C) centroid stage (centroids, entropy, repulsion, inter) in f64.

Inputs go down in fp8-e3m4 (4 mantissa bits; m,y in [0,1) so dynamic range
is tiny) which halves DMA bytes vs f16; all device accumulation is f32.
"""

import numpy as np
import ml_dtypes

N = 16384
F = 64
K = 32
C = 128
NCORES = 8
F_PER_CORE = F // NCORES          # 8
FK = F_PER_CORE * K               # 256 bins per core
NT = N // 128                     # 128 row-tiles

PG = 8                            # n-subtiles per packed super-block
NB = NT // PG                     # 16 super-blocks
YW = C + 4                        # 132: [Y | 1 | ysq_hi | ysq_lo | pad]
YWP = 144                        # y slot padded so every slot is 16B-aligned
PAIRW = 2 * FK + 2 * YWP          # 800: [g(s0) g(s1) y(s0) y(s1)]
BLKW = (PG // 2) * PAIRW          # 3200 cols per super-block
YSQ_SCALE = 16.0                  # keep ysq/16 < 8 so it fits e3m4 (max 15.5)
WARM_MM = 14                      # PE warm-up matmuls (HAM un-throttle)

LAMBDA_ENTROPY = 0.1
LAMBDA_REPULSION = 0.5
LAMBDA_INTER = 0.3
EPS = 1e-8

_NC_CACHE = {}

_NPDT = {
    "f8e3": ml_dtypes.float8_e3m4,
    "f8e4": ml_dtypes.float8_e4m3,
    "f16": np.float16,
}


def _pack_gy(gc: np.ndarray, yext: np.ndarray) -> np.ndarray:
    """gc (N, FK), yext (N, YWP) -> (NB*128, BLKW) consumption-interleaved:
    row p of block b, pair j holds [g(s0)|g(s1)|y(s0)|y(s1)] for subtiles
    s{u} = b*PG + 2j + u, each taking its rows [s*128 + p]."""
    g5 = gc.reshape(NB, PG // 2, 2, 128, FK)
    y5 = yext.reshape(NB, PG // 2, 2, 128, YWP)
    gp = g5.transpose(0, 1, 3, 2, 4).reshape(NB, PG // 2, 128, 2 * FK)
    yp = y5.transpose(0, 1, 3, 2, 4).reshape(NB, PG // 2, 128, 2 * YWP)
    blk = np.concatenate([gp, yp], axis=3)        # (NB, PG//2, 128, PAIRW)
    return np.ascontiguousarray(
        blk.transpose(0, 2, 1, 3).reshape(NB * 128, BLKW)
    )


def _finalize(parts: np.ndarray):
    """parts: (ncores, 128, 2*YW) f64; cols [0:YW] = bins 0-127, [YW:] = 128-255,
    each row [SWY | mass | A_hi | A_lo]."""
    R = parts.reshape(NCORES, 128, 2, YW).transpose(0, 2, 1, 3).reshape(F, K, YW)
    mass_raw = R[..., C]
    bm = mass_raw + EPS
    A = YSQ_SCALE * (R[..., C + 1] + R[..., C + 2])
    cent = R[..., 0:C] / bm[..., None]            # (F,K,C)
    c_sq = (cent * cent).sum(-1)                  # (F,K)
    disp = (A / bm - c_sq - EPS * c_sq / bm).sum()
    p = bm / N
    ent = (p * np.log(p + EPS)).sum()
    nd = ((cent[:, :-1] - cent[:, 1:]) ** 2).sum(-1)
    rep = np.exp(-nd).sum()
    dots = np.einsum('fkc,fjc->fkj', cent, cent)
    pw = c_sq[:, :, None] + c_sq[:, None, :] - 2.0 * dots
    iu, ju = np.triu_indices(K, 1)
    inter = np.exp(-pw[:, iu, ju]).sum() / F
    tot = disp + LAMBDA_ENTROPY * ent + LAMBDA_REPULSION * rep + LAMBDA_INTER * inter
    return tuple(np.float32(v) for v in (tot, disp, ent, rep, inter))


def _build_nc(mode: str):
    import concourse.bacc as bacc
    import concourse.tile as tile
    from concourse import mybir

    f32 = mybir.dt.float32
    fin = {"f8e3": mybir.dt.float8e3, "f8e4": mybir.dt.float8e4,
           "f16": mybir.dt.float16}[mode]

    nc = bacc.Bacc("TRN2", target_bir_lowering=False, debug=False,
                   enable_asserts=False, enable_partition_id=False)
    gy_dram = nc.dram_tensor("gy", (NB * 128, BLKW), fin, kind="ExternalInput").ap()
    out_dram = nc.dram_tensor("out", (128, 2 * YW), f32, kind="ExternalOutput").ap()

    with tile.TileContext(nc) as tc:
        with (
            tc.tile_pool(name="singles", bufs=1) as singles,
            tc.tile_pool(name="gpool", bufs=NB) as gpool,
            tc.tile_pool(name="res", bufs=1) as res,
            tc.tile_pool(name="psacc", bufs=1, space="PSUM") as psacc,
            tc.tile_pool(name="pswarm", bufs=1, space="PSUM") as pswarm,
        ):
            # PE warm-up: dependency-free matmuls keep the array busy while
            # the first DMAs land, so HAM reaches K=8/8 before real work.
            wsb = singles.tile([128, 64], f32)
            nc.vector.memset(wsb, 0.0)
            wps = pswarm.tile([64, 64], f32)
            for _ in range(WARM_MM):
                nc.tensor.matmul(wps, wsb, wsb, start=True, stop=True)

            gyts = [gpool.tile([128, BLKW], fin, name=f"gy{b}", tag="gy")
                    for b in range(NB)]

            # single ring, strict consumption order; the first piece is one
            # subtile-pair so the first matmuls' data (and its completion
            # receipt) arrive as early as possible
            for lo, hi in ((0, PAIRW), (PAIRW, BLKW)):
                nc.sync.dma_start(out=gyts[0][:, lo:hi],
                                  in_=gy_dram[0:128, lo:hi])
            for b in range(1, NB):
                nc.sync.dma_start(out=gyts[b],
                                  in_=gy_dram[b * 128:(b + 1) * 128, :])

            # phase 1: ps_h[fk, 132] += G_half_s^T @ yext_s over 128 subtiles
            ps0 = psacc.tile([128, YW], f32)
            ps1 = psacc.tile([128, YW], f32)
            for b in range(NB):
                for t in range(PG):
                    s = b * PG + t
                    j, u = t // 2, t % 2
                    gcol = j * PAIRW + u * FK
                    ycol = j * PAIRW + 2 * FK + u * YWP
                    rhs = gyts[b][:, ycol:ycol + YW]
                    st, sp = (s == 0), (s == NT - 1)
                    nc.tensor.matmul(
                        ps0, gyts[b][:, gcol:gcol + 128], rhs, start=st, stop=sp)
                    nc.tensor.matmul(
                        ps1, gyts[b][:, gcol + 128:gcol + FK], rhs, start=st, stop=sp)

            # drain: psum -> sbuf on two engines, out halves on both rings
            sbout = res.tile([128, 2 * YW], f32)
            nc.scalar.copy(sbout[:, 0:YW], ps0)
            nc.vector.tensor_copy(sbout[:, YW:2 * YW], ps1)
            nc.sync.dma_start(out=out_dram[:, 0:YW], in_=sbout[:, 0:YW])
            nc.scalar.dma_start(out=out_dram[:, YW:2 * YW], in_=sbout[:, YW:2 * YW])

    nc.compile()
    return nc


def get_nc(mode: str = "f8e3"):
    if mode not in _NC_CACHE:
        _NC_CACHE[mode] = _build_nc(mode)
    return _NC_CACHE[mode]


def kernel(membership: np.ndarray, teacher_preds: np.ndarray, _trace: bool = False,
           _mode: str = "f8e3"):
    from concourse.bass_utils import run_bass_kernel_spmd

    npdt = _NPDT[_mode]
    y32 = np.asarray(teacher_preds, dtype=np.float32)
    ysq = (y32.astype(np.float64) ** 2).sum(axis=1) / YSQ_SCALE   # exact, host
    hi = ysq.astype(np.float32).astype(npdt)
    lo = (ysq - hi.astype(np.float64)).astype(np.float32).astype(npdt)
    yext = np.zeros((N, YWP), dtype=npdt)
    yext[:, 0:C] = y32.astype(npdt)
    yext[:, C] = np.float32(1.0)
    yext[:, C + 1] = hi
    yext[:, C + 2] = lo

    m = np.asarray(membership, dtype=np.float32).reshape(N, F * K).astype(npdt)

    nc = get_nc(_mode)
    in_maps = []
    for i in range(NCORES):
        in_maps.append({
            "gy": _pack_gy(m[:, i * FK:(i + 1) * FK], yext),
        })
    res = run_bass_kernel_spmd(
        nc, in_maps, core_ids=list(range(NCORES)), trace=_trace,
    )
    parts = np.stack(
        [np.asarray(res.results[i]["out"], dtype=np.float64) for i in range(NCORES)]
    )
    out = _finalize(parts)
    if _trace:
        return out, res
    return out


if __name__ == "__main__":
    rng = np.random.default_rng(0)
    mem = rng.random((N, F, K), dtype=np.float32)
    tp = rng.random((N, C), dtype=np.float32)
    print(kernel(mem, tp))


# revision 32
# speedup vs baseline: 1.0488x; 1.0488x over previous
"""DispersionLoss kernel for Trainium2 (8 NeuronCores, Bass/Tile).

Reference computation (N=16384, F=64, K=32, C=128):
    bin_mass[f,k]  = sum_n m[n,f,k] + EPS
    SWY[f,k,c]     = sum_n m[n,f,k] * y[n,c]
    cent[f,k,c]    = SWY / bin_mass
    loss_dispersion= sum_fk ( A/bin_mass - c_sq - EPS*c_sq/bin_mass )
        where A[f,k] = sum_n m[n,f,k]*|y_n|^2   (algebraic expansion: the
        cross term sum_n m*cross equals bin_mass*c_sq exactly)
    loss_entropy   = sum_fk p*log(p+EPS), p = bin_mass/N
    loss_repulsion = sum_f sum_k exp(-|cent[f,k]-cent[f,k+1]|^2)
    loss_inter     = sum_f sum_{k<j} exp(-|cent[f,k]-cent[f,j]|^2) / F

Sharding: over F (8 features per core); every loss term decomposes per-f.

Device phase (the N-reduction, 99.9% of FLOPs): per 128-row subtile s, the
G half-tiles (128n x 128fk) are the STATIONARY operands and the moving
operand is yext_s = [Y | 1 | ysq_hi | ysq_lo] (128n x 132).  Two matmuls
per subtile accumulate ps_h[fk, 132] = [SWY | mass | A_hi | A_lo] directly
in bin-major layout.  y_sq is computed exactly on the host from f32 y and
shipped split into fp8 hi+lo parts so its quantization error is ~1e-4.

G and Y are host-interleaved into ONE dram tensor in exact consumption
order, so a single DMA ring delivers them with no cross-stream contention:
block b = [g(s0) g(s1) y(s0) y(s1)] x 4 subtile-pairs (all slots 16B-
aligned for fast weight load), 410 KB, whose transfer time (~1.15 us)
matches the matmul consumption per block.

The per-core (256 x 132) f32 result is DMA'd out; the host finishes the
tiny (F,K,C) centroid stage (centroids, entropy, repulsion, inter) in f64.

Inputs go down in fp8-e3m4 (4 mantissa bits; m,y in [0,1) so dynamic range
is tiny) which halves DMA bytes vs f16; all device accumulation is f32.
"""

import numpy as np
import ml_dtypes

N = 16384
F = 64
K = 32
C = 128
NCORES = 8
F_PER_CORE = F // NCORES          # 8
FK = F_PER_CORE * K               # 256 bins per core
NT = N // 128                     # 128 row-tiles

PG = 8                            # n-subtiles per packed super-block
NB = NT // PG                     # 16 super-blocks
YW = C + 4                        # 132: [Y | 1 | ysq_hi | ysq_lo | pad]
YWP = 144                        # y slot padded so every slot is 16B-aligned
PAIRW = 2 * FK + 2 * YWP          # 800: [g(s0) g(s1) y(s0) y(s1)]
BLKW = (PG // 2) * PAIRW          # 3200 cols per super-block
YSQ_SCALE = 16.0                  # keep ysq/16 < 8 so it fits e3m4 (max 15.5)
WARM_MM = 14                      # PE warm-up matmuls (HAM un-throttle)

LAMBDA_ENTROPY = 0.1
LAMBDA_REPULSION = 0.5
LAMBDA_INTER = 0.3
EPS = 1e-8

_NC_CACHE = {}

_NPDT = {
    "f8e3": ml_dtypes.float8_e3m4,
    "f8e4": ml_dtypes.float8_e4m3,
    "f16": np.float16,
}


def _pack_gy(gc: np.ndarray, yext: np.ndarray) -> np.ndarray:
    """gc (N, FK), yext (N, YWP) -> (NB*128, BLKW) consumption-interleaved:
    row p of block b, pair j holds [g(s0)|g(s1)|y(s0)|y(s1)] for subtiles
    s{u} = b*PG + 2j + u, each taking its rows [s*128 + p]."""
    g5 = gc.reshape(NB, PG // 2, 2, 128, FK)
    y5 = yext.reshape(NB, PG // 2, 2, 128, YWP)
    gp = g5.transpose(0, 1, 3, 2, 4).reshape(NB, PG // 2, 128, 2 * FK)
    yp = y5.transpose(0, 1, 3, 2, 4).reshape(NB, PG // 2, 128, 2 * YWP)
    blk = np.concatenate([gp, yp], axis=3)        # (NB, PG//2, 128, PAIRW)
    return np.ascontiguousarray(
        blk.transpose(0, 2, 1, 3).reshape(NB * 128, BLKW)
    )


def _finalize(parts: np.ndarray):
    """parts: (ncores, 128, 2*YW) f64; cols [0:YW] = bins 0-127, [YW:] = 128-255,
    each row [SWY | mass | A_hi | A_lo]."""
    R = parts.reshape(NCORES, 128, 2, YW).transpose(0, 2, 1, 3).reshape(F, K, YW)
    mass_raw = R[..., C]
    bm = mass_raw + EPS
    A = YSQ_SCALE * (R[..., C + 1] + R[..., C + 2])
    cent = R[..., 0:C] / bm[..., None]            # (F,K,C)
    c_sq = (cent * cent).sum(-1)                  # (F,K)
    disp = (A / bm - c_sq - EPS * c_sq / bm).sum()
    p = bm / N
    ent = (p * np.log(p + EPS)).sum()
    nd = ((cent[:, :-1] - cent[:, 1:]) ** 2).sum(-1)
    rep = np.exp(-nd).sum()
    dots = np.einsum('fkc,fjc->fkj', cent, cent)
    pw = c_sq[:, :, None] + c_sq[:, None, :] - 2.0 * dots
    iu, ju = np.triu_indices(K, 1)
    inter = np.exp(-pw[:, iu, ju]).sum() / F
    tot = disp + LAMBDA_ENTROPY * ent + LAMBDA_REPULSION * rep + LAMBDA_INTER * inter
    return tuple(np.float32(v) for v in (tot, disp, ent, rep, inter))


def _build_nc(mode: str):
    import concourse.bacc as bacc
    import concourse.tile as tile
    from concourse import mybir

    f32 = mybir.dt.float32
    fin = {"f8e3": mybir.dt.float8e3, "f8e4": mybir.dt.float8e4,
           "f16": mybir.dt.float16}[mode]

    nc = bacc.Bacc("TRN2", target_bir_lowering=False, debug=False,
                   enable_asserts=False, enable_partition_id=False)
    gy_dram = nc.dram_tensor("gy", (NB * 128, BLKW), fin, kind="ExternalInput").ap()
    out_dram = nc.dram_tensor("out", (128, 2 * YW), f32, kind="ExternalOutput").ap()

    with tile.TileContext(nc) as tc:
        with (
            tc.tile_pool(name="singles", bufs=1) as singles,
            tc.tile_pool(name="gpool", bufs=NB) as gpool,
            tc.tile_pool(name="res", bufs=1) as res,
            tc.tile_pool(name="psacc", bufs=1, space="PSUM") as psacc,
            tc.tile_pool(name="pswarm", bufs=1, space="PSUM") as pswarm,
        ):
            # PE warm-up: dependency-free matmuls keep the array busy while
            # the first DMAs land, so HAM reaches K=8/8 before real work.
            wsb = singles.tile([128, 64], f32)
            nc.vector.memset(wsb, 0.0)
            wps = pswarm.tile([64, 64], f32)
            for _ in range(WARM_MM):
                nc.tensor.matmul(wps, wsb, wsb, start=True, stop=True)

            gyts = [gpool.tile([128, BLKW], fin, name=f"gy{b}", tag="gy")
                    for b in range(NB)]

            # single ring, strict consumption order; the first piece is one
            # subtile-pair so the first matmuls' data (and its completion
            # receipt) arrive as early as possible
            for lo, hi in ((0, PAIRW), (PAIRW, BLKW)):
                nc.sync.dma_start(out=gyts[0][:, lo:hi],
                                  in_=gy_dram[0:128, lo:hi])
            for b in range(1, NB):
                nc.sync.dma_start(out=gyts[b],
                                  in_=gy_dram[b * 128:(b + 1) * 128, :])

            # phase 1: ps_h[fk, 132] += G_half_s^T @ yext_s over 128 subtiles.
            # In the last block all ps0 matmuls run before all ps1 matmuls so
            # ps0's drain (copy + out-DMA + HBM receipt) overlaps ps1's tail.
            ps0 = psacc.tile([128, YW], f32)
            ps1 = psacc.tile([128, YW], f32)

            def cols(t):
                j, u = t // 2, t % 2
                gcol = j * PAIRW + u * FK
                ycol = j * PAIRW + 2 * FK + u * YWP
                return gcol, ycol

            def mm(half, b, t):
                s = b * PG + t
                gcol, ycol = cols(t)
                rhs = gyts[b][:, ycol:ycol + YW]
                st, sp = (s == 0), (s == NT - 1)
                ps, off = (ps0, gcol) if half == 0 else (ps1, gcol + 128)
                nc.tensor.matmul(ps, gyts[b][:, off:off + 128], rhs,
                                 start=st, stop=sp)

            for b in range(NB - 1):
                for t in range(PG):
                    mm(0, b, t)
                    mm(1, b, t)
            for t in range(PG):
                mm(0, NB - 1, t)
            sbout = res.tile([128, 2 * YW], f32)
            nc.scalar.copy(sbout[:, 0:YW], ps0)
            nc.sync.dma_start(out=out_dram[:, 0:YW], in_=sbout[:, 0:YW])
            for t in range(PG):
                mm(1, NB - 1, t)
            nc.vector.tensor_copy(sbout[:, YW:2 * YW], ps1)
            nc.scalar.dma_start(out=out_dram[:, YW:2 * YW], in_=sbout[:, YW:2 * YW])

    nc.compile()
    return nc


def get_nc(mode: str = "f8e3"):
    if mode not in _NC_CACHE:
        _NC_CACHE[mode] = _build_nc(mode)
    return _NC_CACHE[mode]


def kernel(membership: np.ndarray, teacher_preds: np.ndarray, _trace: bool = False,
           _mode: str = "f8e3"):
    from concourse.bass_utils import run_bass_kernel_spmd

    npdt = _NPDT[_mode]
    y32 = np.asarray(teacher_preds, dtype=np.float32)
    ysq = (y32.astype(np.float64) ** 2).sum(axis=1) / YSQ_SCALE   # exact, host
    hi = ysq.astype(np.float32).astype(npdt)
    lo = (ysq - hi.astype(np.float64)).astype(np.float32).astype(npdt)
    yext = np.zeros((N, YWP), dtype=npdt)
    yext[:, 0:C] = y32.astype(npdt)
    yext[:, C] = np.float32(1.0)
    yext[:, C + 1] = hi
    yext[:, C + 2] = lo

    m = np.asarray(membership, dtype=np.float32).reshape(N, F * K).astype(npdt)

    nc = get_nc(_mode)
    in_maps = []
    for i in range(NCORES):
        in_maps.append({
            "gy": _pack_gy(m[:, i * FK:(i + 1) * FK], yext),
        })
    res = run_bass_kernel_spmd(
        nc, in_maps, core_ids=list(range(NCORES)), trace=_trace,
    )
    parts = np.stack(
        [np.asarray(res.results[i]["out"], dtype=np.float64) for i in range(NCORES)]
    )
    out = _finalize(parts)
    if _trace:
        return out, res
    return out


if __name__ == "__main__":
    rng = np.random.default_rng(0)
    mem = rng.random((N, F, K), dtype=np.float32)
    tp = rng.random((N, C), dtype=np.float32)
    print(kernel(mem, tp))


# revision 33
# speedup vs baseline: 1.0578x; 1.0085x over previous
"""DispersionLoss kernel for Trainium2 (8 NeuronCores, Bass/Tile).

Reference computation (N=16384, F=64, K=32, C=128):
    bin_mass[f,k]  = sum_n m[n,f,k] + EPS
    SWY[f,k,c]     = sum_n m[n,f,k] * y[n,c]
    cent[f,k,c]    = SWY / bin_mass
    loss_dispersion= sum_fk ( A/bin_mass - c_sq - EPS*c_sq/bin_mass )
        where A[f,k] = sum_n m[n,f,k]*|y_n|^2   (algebraic expansion: the
        cross term sum_n m*cross equals bin_mass*c_sq exactly)
    loss_entropy   = sum_fk p*log(p+EPS), p = bin_mass/N
    loss_repulsion = sum_f sum_k exp(-|cent[f,k]-cent[f,k+1]|^2)
    loss_inter     = sum_f sum_{k<j} exp(-|cent[f,k]-cent[f,j]|^2) / F

Sharding: over F (8 features per core); every loss term decomposes per-f.

Device phase (the N-reduction, 99.9% of FLOPs): per 128-row subtile s, the
G half-tiles (128n x 128fk) are the STATIONARY operands and the moving
operand is yext_s = [Y | 1 | ysq_hi | ysq_lo] (128n x 132).  Two matmuls
per subtile accumulate ps_h[fk, 132] = [SWY | mass | A_hi | A_lo] directly
in bin-major layout.  y_sq is computed exactly on the host from f32 y and
shipped split into fp8 hi+lo parts so its quantization error is ~1e-4.

G and Y are host-interleaved into ONE dram tensor in exact consumption
order, so a single DMA ring delivers them with no cross-stream contention:
block b = [g(s0) g(s1) y(s0) y(s1)] x 4 subtile-pairs (all slots 16B-
aligned for fast weight load), 410 KB, whose transfer time (~1.15 us)
matches the matmul consumption per block.

The per-core (256 x 132) f32 result is DMA'd out; the host finishes the
tiny (F,K,C) centroid stage (centroids, entropy, repulsion, inter) in f64.

Inputs go down in fp8-e3m4 (4 mantissa bits; m,y in [0,1) so dynamic range
is tiny) which halves DMA bytes vs f16; all device accumulation is f32.
"""

import numpy as np
import ml_dtypes

N = 16384
F = 64
K = 32
C = 128
NCORES = 8
F_PER_CORE = F // NCORES          # 8
FK = F_PER_CORE * K               # 256 bins per core
NT = N // 128                     # 128 row-tiles

PG = 8                            # n-subtiles per packed super-block
NB = NT // PG                     # 16 super-blocks
YW = C + 4                        # 132: [Y | 1 | ysq_hi | ysq_lo | pad]
YWP = 136                        # y slot padded so g slots stay 16B-aligned
PAIRW = 2 * FK + 2 * YWP          # 800: [g(s0) g(s1) y(s0) y(s1)]
BLKW = (PG // 2) * PAIRW          # 3200 cols per super-block
YSQ_SCALE = 16.0                  # keep ysq/16 < 8 so it fits e3m4 (max 15.5)
WARM_MM = 14                      # PE warm-up matmuls (HAM un-throttle)

LAMBDA_ENTROPY = 0.1
LAMBDA_REPULSION = 0.5
LAMBDA_INTER = 0.3
EPS = 1e-8

_NC_CACHE = {}

_NPDT = {
    "f8e3": ml_dtypes.float8_e3m4,
    "f8e4": ml_dtypes.float8_e4m3,
    "f16": np.float16,
}


def _pack_gy(gc: np.ndarray, yext: np.ndarray) -> np.ndarray:
    """gc (N, FK), yext (N, YWP) -> (NB*128, BLKW) consumption-interleaved:
    row p of block b, pair j holds [g(s0)|g(s1)|y(s0)|y(s1)] for subtiles
    s{u} = b*PG + 2j + u, each taking its rows [s*128 + p]."""
    g5 = gc.reshape(NB, PG // 2, 2, 128, FK)
    y5 = yext.reshape(NB, PG // 2, 2, 128, YWP)
    gp = g5.transpose(0, 1, 3, 2, 4).reshape(NB, PG // 2, 128, 2 * FK)
    yp = y5.transpose(0, 1, 3, 2, 4).reshape(NB, PG // 2, 128, 2 * YWP)
    blk = np.concatenate([gp, yp], axis=3)        # (NB, PG//2, 128, PAIRW)
    return np.ascontiguousarray(
        blk.transpose(0, 2, 1, 3).reshape(NB * 128, BLKW)
    )


def _finalize(parts: np.ndarray):
    """parts: (ncores, 128, 2*YW) f64; cols [0:YW] = bins 0-127, [YW:] = 128-255,
    each row [SWY | mass | A_hi | A_lo]."""
    R = parts.reshape(NCORES, 128, 2, YW).transpose(0, 2, 1, 3).reshape(F, K, YW)
    mass_raw = R[..., C]
    bm = mass_raw + EPS
    A = YSQ_SCALE * (R[..., C + 1] + R[..., C + 2])
    cent = R[..., 0:C] / bm[..., None]            # (F,K,C)
    c_sq = (cent * cent).sum(-1)                  # (F,K)
    disp = (A / bm - c_sq - EPS * c_sq / bm).sum()
    p = bm / N
    ent = (p * np.log(p + EPS)).sum()
    nd = ((cent[:, :-1] - cent[:, 1:]) ** 2).sum(-1)
    rep = np.exp(-nd).sum()
    dots = np.einsum('fkc,fjc->fkj', cent, cent)
    pw = c_sq[:, :, None] + c_sq[:, None, :] - 2.0 * dots
    iu, ju = np.triu_indices(K, 1)
    inter = np.exp(-pw[:, iu, ju]).sum() / F
    tot = disp + LAMBDA_ENTROPY * ent + LAMBDA_REPULSION * rep + LAMBDA_INTER * inter
    return tuple(np.float32(v) for v in (tot, disp, ent, rep, inter))


def _build_nc(mode: str):
    import concourse.bacc as bacc
    import concourse.tile as tile
    from concourse import mybir

    f32 = mybir.dt.float32
    fin = {"f8e3": mybir.dt.float8e3, "f8e4": mybir.dt.float8e4,
           "f16": mybir.dt.float16}[mode]

    nc = bacc.Bacc("TRN2", target_bir_lowering=False, debug=False,
                   enable_asserts=False, enable_partition_id=False)
    gy_dram = nc.dram_tensor("gy", (NB * 128, BLKW), fin, kind="ExternalInput").ap()
    out_dram = nc.dram_tensor("out", (128, 2 * YW), f32, kind="ExternalOutput").ap()

    with tile.TileContext(nc) as tc:
        with (
            tc.tile_pool(name="singles", bufs=1) as singles,
            tc.tile_pool(name="gpool", bufs=NB) as gpool,
            tc.tile_pool(name="res", bufs=1) as res,
            tc.tile_pool(name="psacc", bufs=1, space="PSUM") as psacc,
            tc.tile_pool(name="pswarm", bufs=1, space="PSUM") as pswarm,
        ):
            # PE warm-up: dependency-free matmuls keep the array busy while
            # the first DMAs land, so HAM reaches K=8/8 before real work.
            wsb = singles.tile([128, 64], f32)
            nc.vector.memset(wsb, 0.0)
            wps = pswarm.tile([64, 64], f32)
            for _ in range(WARM_MM):
                nc.tensor.matmul(wps, wsb, wsb, start=True, stop=True)

            gyts = [gpool.tile([128, BLKW], fin, name=f"gy{b}", tag="gy")
                    for b in range(NB)]

            # single ring, strict consumption order; the first piece is one
            # subtile-pair so the first matmuls' data (and its completion
            # receipt) arrive as early as possible
            for lo, hi in ((0, PAIRW), (PAIRW, BLKW)):
                nc.sync.dma_start(out=gyts[0][:, lo:hi],
                                  in_=gy_dram[0:128, lo:hi])
            for b in range(1, NB):
                nc.sync.dma_start(out=gyts[b],
                                  in_=gy_dram[b * 128:(b + 1) * 128, :])

            # phase 1: ps_h[fk, 132] += G_half_s^T @ yext_s over 128 subtiles.
            # In the last block all ps0 matmuls run before all ps1 matmuls so
            # ps0's drain (copy + out-DMA + HBM receipt) overlaps ps1's tail.
            ps0 = psacc.tile([128, YW], f32)
            ps1 = psacc.tile([128, YW], f32)

            def cols(t):
                j, u = t // 2, t % 2
                gcol = j * PAIRW + u * FK
                ycol = j * PAIRW + 2 * FK + u * YWP
                return gcol, ycol

            def mm(half, b, t):
                s = b * PG + t
                gcol, ycol = cols(t)
                rhs = gyts[b][:, ycol:ycol + YW]
                st, sp = (s == 0), (s == NT - 1)
                ps, off = (ps0, gcol) if half == 0 else (ps1, gcol + 128)
                nc.tensor.matmul(ps, gyts[b][:, off:off + 128], rhs,
                                 start=st, stop=sp)

            for b in range(NB - 1):
                for t in range(PG):
                    mm(0, b, t)
                    mm(1, b, t)
            for t in range(PG):
                mm(0, NB - 1, t)
            sbout = res.tile([128, 2 * YW], f32)
            nc.scalar.copy(sbout[:, 0:YW], ps0)
            nc.sync.dma_start(out=out_dram[:, 0:YW], in_=sbout[:, 0:YW])
            for t in range(PG):
                mm(1, NB - 1, t)
            nc.vector.tensor_copy(sbout[:, YW:2 * YW], ps1)
            nc.scalar.dma_start(out=out_dram[:, YW:2 * YW], in_=sbout[:, YW:2 * YW])

    nc.compile()
    return nc


def get_nc(mode: str = "f8e3"):
    if mode not in _NC_CACHE:
        _NC_CACHE[mode] = _build_nc(mode)
    return _NC_CACHE[mode]


def kernel(membership: np.ndarray, teacher_preds: np.ndarray, _trace: bool = False,
           _mode: str = "f8e3"):
    from concourse.bass_utils import run_bass_kernel_spmd

    npdt = _NPDT[_mode]
    y32 = np.asarray(teacher_preds, dtype=np.float32)
    ysq = (y32.astype(np.float64) ** 2).sum(axis=1) / YSQ_SCALE   # exact, host
    hi = ysq.astype(np.float32).astype(npdt)
    lo = (ysq - hi.astype(np.float64)).astype(np.float32).astype(npdt)
    yext = np.zeros((N, YWP), dtype=npdt)
    yext[:, 0:C] = y32.astype(npdt)
    yext[:, C] = np.float32(1.0)
    yext[:, C + 1] = hi
    yext[:, C + 2] = lo

    m = np.asarray(membership, dtype=np.float32).reshape(N, F * K).astype(npdt)

    nc = get_nc(_mode)
    in_maps = []
    for i in range(NCORES):
        in_maps.append({
            "gy": _pack_gy(m[:, i * FK:(i + 1) * FK], yext),
        })
    res = run_bass_kernel_spmd(
        nc, in_maps, core_ids=list(range(NCORES)), trace=_trace,
    )
    parts = np.stack(
        [np.asarray(res.results[i]["out"], dtype=np.float64) for i in range(NCORES)]
    )
    out = _finalize(parts)
    if _trace:
        return out, res
    return out


if __name__ == "__main__":
    rng = np.random.default_rng(0)
    mem = rng.random((N, F, K), dtype=np.float32)
    tp = rng.random((N, C), dtype=np.float32)
    print(kernel(mem, tp))


# revision 35
# speedup vs baseline: 1.0992x; 1.0392x over previous
"""DispersionLoss kernel for Trainium2 (8 NeuronCores, Bass/Tile).

Reference computation (N=16384, F=64, K=32, C=128):
    bin_mass[f,k]  = sum_n m[n,f,k] + EPS
    SWY[f,k,c]     = sum_n m[n,f,k] * y[n,c]
    cent[f,k,c]    = SWY / bin_mass
    loss_dispersion= sum_fk ( A/bin_mass - c_sq - EPS*c_sq/bin_mass )
        where A[f,k] = sum_n m[n,f,k]*|y_n|^2   (algebraic expansion: the
        cross term sum_n m*cross equals bin_mass*c_sq exactly)
    loss_entropy   = sum_fk p*log(p+EPS), p = bin_mass/N
    loss_repulsion = sum_f sum_k exp(-|cent[f,k]-cent[f,k+1]|^2)
    loss_inter     = sum_f sum_{k<j} exp(-|cent[f,k]-cent[f,j]|^2) / F

Sharding: over F (8 features per core); every loss term decomposes per-f.

Device phase (the N-reduction, 99.9% of FLOPs): per 128-row subtile s, the
G half-tiles (128n x 128fk) are the STATIONARY operands and the moving
operand is yext_s = [Y | 1 | ysq_hi | ysq_lo] (128n x 132).  Two matmuls
per subtile accumulate ps_h[fk, 132] = [SWY | mass | A_hi | A_lo] directly
in bin-major layout.  y_sq is computed exactly on the host from f32 y and
shipped split into fp8 hi+lo parts so its quantization error is ~1e-4.

G and Y are host-interleaved into ONE dram tensor in exact consumption
order, so a single DMA ring delivers them with no cross-stream contention:
block b = [g(s0) g(s1) y(s0) y(s1)] x 4 subtile-pairs (all slots 16B-
aligned for fast weight load), 410 KB, whose transfer time (~1.15 us)
matches the matmul consumption per block.

The per-core (256 x 132) f32 result is DMA'd out; the host finishes the
tiny (F,K,C) centroid stage (centroids, entropy, repulsion, inter) in f64.

Inputs go down in fp8-e3m4 (4 mantissa bits; m,y in [0,1) so dynamic range
is tiny) which halves DMA bytes vs f16; all device accumulation is f32.
"""

import numpy as np
import ml_dtypes

N = 16384
F = 64
K = 32
C = 128
NCORES = 8
F_PER_CORE = F // NCORES          # 8
FK = F_PER_CORE * K               # 256 bins per core
NT = N // 128                     # 128 row-tiles

PG = 8                            # n-subtiles per packed super-block
NB = NT // PG                     # 16 super-blocks
YW = C + 4                        # 132: [Y | 1 | ysq_hi | ysq_lo | pad]
YWP = 136                        # y slot padded so g slots stay 16B-aligned
PAIRW = 2 * FK + 2 * YWP          # 800: [g(s0) g(s1) y(s0) y(s1)]
BLKW = (PG // 2) * PAIRW          # 3200 cols per super-block
YSQ_SCALE = 16.0                  # keep ysq/16 < 8 so it fits e3m4 (max 15.5)
WARM_MM = 14                      # PE warm-up matmuls (HAM un-throttle)

LAMBDA_ENTROPY = 0.1
LAMBDA_REPULSION = 0.5
LAMBDA_INTER = 0.3
EPS = 1e-8

_NC_CACHE = {}

_NPDT = {
    "f8e3": ml_dtypes.float8_e3m4,
    "f8e4": ml_dtypes.float8_e4m3,
    "f16": np.float16,
}


def _pack_gy(gc: np.ndarray, yext: np.ndarray) -> np.ndarray:
    """gc (N, FK), yext (N, YWP) -> (NB*128, BLKW) consumption-interleaved:
    row p of block b, pair j holds [g(s0)|g(s1)|y(s0)|y(s1)] for subtiles
    s{u} = b*PG + 2j + u, each taking its rows [s*128 + p]."""
    g5 = gc.reshape(NB, PG // 2, 2, 128, FK)
    y5 = yext.reshape(NB, PG // 2, 2, 128, YWP)
    gp = g5.transpose(0, 1, 3, 2, 4).reshape(NB, PG // 2, 128, 2 * FK)
    yp = y5.transpose(0, 1, 3, 2, 4).reshape(NB, PG // 2, 128, 2 * YWP)
    blk = np.concatenate([gp, yp], axis=3)        # (NB, PG//2, 128, PAIRW)
    return np.ascontiguousarray(
        blk.transpose(0, 2, 1, 3).reshape(NB * 128, BLKW)
    )


def _finalize(parts: np.ndarray):
    """parts: (ncores, 128, 2*YW) f64; cols [0:YW] = bins 0-127, [YW:] = 128-255,
    each row [SWY | mass | A_hi | A_lo]."""
    R = parts.reshape(NCORES, 128, 2, YW).transpose(0, 2, 1, 3).reshape(F, K, YW)
    mass_raw = R[..., C]
    bm = mass_raw + EPS
    A = YSQ_SCALE * (R[..., C + 1] + R[..., C + 2])
    cent = R[..., 0:C] / bm[..., None]            # (F,K,C)
    c_sq = (cent * cent).sum(-1)                  # (F,K)
    disp = (A / bm - c_sq - EPS * c_sq / bm).sum()
    p = bm / N
    ent = (p * np.log(p + EPS)).sum()
    nd = ((cent[:, :-1] - cent[:, 1:]) ** 2).sum(-1)
    rep = np.exp(-nd).sum()
    dots = np.einsum('fkc,fjc->fkj', cent, cent)
    pw = c_sq[:, :, None] + c_sq[:, None, :] - 2.0 * dots
    iu, ju = np.triu_indices(K, 1)
    inter = np.exp(-pw[:, iu, ju]).sum() / F
    tot = disp + LAMBDA_ENTROPY * ent + LAMBDA_REPULSION * rep + LAMBDA_INTER * inter
    return tuple(np.float32(v) for v in (tot, disp, ent, rep, inter))


def _build_nc(mode: str):
    import concourse.bacc as bacc
    import concourse.tile as tile
    from concourse import mybir

    f32 = mybir.dt.float32
    fin = {"f8e3": mybir.dt.float8e3, "f8e4": mybir.dt.float8e4,
           "f16": mybir.dt.float16}[mode]

    nc = bacc.Bacc("TRN2", target_bir_lowering=False, debug=False,
                   enable_asserts=False, enable_partition_id=False)
    gy_dram = nc.dram_tensor("gy", (NB * 128, BLKW), fin, kind="ExternalInput").ap()
    out_dram = nc.dram_tensor("out", (128, 2 * YW), f32, kind="ExternalOutput").ap()

    with tile.TileContext(nc) as tc:
        with (
            tc.tile_pool(name="singles", bufs=1) as singles,
            tc.tile_pool(name="gpool", bufs=NB) as gpool,
            tc.tile_pool(name="res", bufs=1) as res,
            tc.tile_pool(name="psacc", bufs=1, space="PSUM") as psacc,
            tc.tile_pool(name="pswarm", bufs=1, space="PSUM") as pswarm,
        ):
            # PE warm-up: dependency-free matmuls keep the array busy while
            # the first DMAs land, so HAM reaches K=8/8 before real work.
            wsb = singles.tile([128, 64], f32)
            nc.vector.memset(wsb, 0.0)
            wps = pswarm.tile([64, 64], f32)
            for _ in range(WARM_MM):
                nc.tensor.matmul(wps, wsb, wsb, start=True, stop=True)

            gyts = [gpool.tile([128, BLKW], fin, name=f"gy{b}", tag="gy")
                    for b in range(NB)]

            # single ring, strict consumption order; the first piece is one
            # subtile-pair so the first matmuls' data (and its completion
            # receipt) arrive as early as possible
            for lo, hi in ((0, PAIRW), (PAIRW, BLKW)):
                nc.sync.dma_start(out=gyts[0][:, lo:hi],
                                  in_=gy_dram[0:128, lo:hi])
            for b in range(1, NB):
                nc.sync.dma_start(out=gyts[b],
                                  in_=gy_dram[b * 128:(b + 1) * 128, :])

            # phase 1: ps_h[fk, 132] += G_half_s^T @ yext_s over 128 subtiles.
            # In the last block all ps0 matmuls run before all ps1 matmuls so
            # ps0's drain (copy + out-DMA + HBM receipt) overlaps ps1's tail.
            ps0 = psacc.tile([128, YW], f32)
            ps1 = psacc.tile([128, YW], f32)

            def cols(t):
                j, u = t // 2, t % 2
                gcol = j * PAIRW + u * FK
                ycol = j * PAIRW + 2 * FK + u * YWP
                return gcol, ycol

            def mm(half, b, t):
                s = b * PG + t
                gcol, ycol = cols(t)
                rhs = gyts[b][:, ycol:ycol + YW]
                st, sp = (s == 0), (s == NT - 1)
                ps, off = (ps0, gcol) if half == 0 else (ps1, gcol + 128)
                nc.tensor.matmul(ps, gyts[b][:, off:off + 128], rhs,
                                 start=st, stop=sp)

            for b in range(NB - 1):
                for t in range(PG):
                    mm(0, b, t)
                    mm(1, b, t)
            for t in range(PG):
                mm(0, NB - 1, t)
            sbout = res.tile([128, 2 * YW], f32)
            nc.scalar.copy(sbout[:, 0:YW], ps0)
            nc.sync.dma_start(out=out_dram[:, 0:YW], in_=sbout[:, 0:YW])
            for t in range(PG):
                mm(1, NB - 1, t)
            nc.vector.tensor_copy(sbout[:, YW:2 * YW], ps1)
            nc.scalar.dma_start(out=out_dram[:, YW:2 * YW], in_=sbout[:, YW:2 * YW])

    nc.compile()
    return nc


def get_nc(mode: str = "f8e3"):
    if mode not in _NC_CACHE:
        _NC_CACHE[mode] = _build_nc(mode)
    return _NC_CACHE[mode]


def kernel(membership: np.ndarray, teacher_preds: np.ndarray, _trace: bool = False,
           _mode: str = "f8e3"):
    from concourse.bass_utils import run_bass_kernel_spmd

    npdt = _NPDT[_mode]
    y32 = np.asarray(teacher_preds, dtype=np.float32)
    ysq = (y32.astype(np.float64) ** 2).sum(axis=1) / YSQ_SCALE   # exact, host
    hi = ysq.astype(np.float32).astype(npdt)
    lo = (ysq - hi.astype(np.float64)).astype(np.float32).astype(npdt)
    yext = np.zeros((N, YWP), dtype=npdt)
    yext[:, 0:C] = y32.astype(npdt)
    yext[:, C] = np.float32(1.0)
    yext[:, C + 1] = hi
    yext[:, C + 2] = lo

    m = np.asarray(membership, dtype=np.float32).reshape(N, F * K).astype(npdt)

    nc = get_nc(_mode)
    in_maps = []
    for i in range(NCORES):
        in_maps.append({
            "gy": _pack_gy(m[:, i * FK:(i + 1) * FK], yext),
        })
    res = run_bass_kernel_spmd(
        nc, in_maps, core_ids=list(range(NCORES)), trace=_trace,
    )
    parts = np.stack(
        [np.asarray(res.results[i]["out"], dtype=np.float64) for i in range(NCORES)]
    )
    out = _finalize(parts)
    if _trace:
        return out, res
    return out


if __name__ == "__main__":
    rng = np.random.default_rng(0)
    mem = rng.random((N, F, K), dtype=np.float32)
    tp = rng.random((N, C), dtype=np.float32)
    print(kernel(mem, tp))
